# revision 34
# baseline (speedup 1.0000x reference)
"""Multi-head self-attention (B=4, T=2048, D=1024, H=16) on 8 TRN2 NeuronCores.

Sharding: tensor-parallel over heads. Core c owns heads (2c, 2c+1):
  - W_Q/W_K/W_V rows [128c, 128c+128) -> per-core q/k/v of shape [T*B, 128]
  - causal attention for its 2 heads
  - partial output projection through W_O columns [128c, 128c+128)
Host sums the 8 partial outputs (the row-parallel W_O reduction).

v3: all matmuls bf16. (fp8 was explored exhaustively: 1-chain fp8-DR
projections are 2x faster but 2.7-4% output error; scaled 3-chain
residual recovers 0.3% error but on real HW a DoubleRow matmul streams
at the same cycles-per-column as bf16 — the 2x is contraction per
instruction — so 3 chains cost 1.5x bf16. AV with fp8 v is 2.4% error:
peaked softmax rows copy single fp8-rounded v elements. All over the 2%
budget.)
Softmax normalization is per q-chunk: the AV stationary matrix carries a
ones column at position 0 so PSUM partition 0 accumulates the softmax
denominator (the HW reciprocal ucode only honors base partition 0), v
lives at M-columns 64-127 (engine partition accesses must be naturally
aligned); DVE reciprocal_approx_fast + f32 GpSimd partition_broadcast +
one DVE multiply straight out of PSUM into hoT (no LN/EXP trick, no
gather, no staging copies).
"""

import os
import sys

import numpy as np

if "/opt/trn_rl_repo" not in sys.path:
    sys.path.insert(0, "/opt/trn_rl_repo")

import ml_dtypes

B, T, D, NH, DH = 4, 2048, 1024, 16, 64
NT = B * T          # 8192 tokens
MT = D // 128       # 8 model-dim tiles
MP = MT // 2        # 4 model-dim tile PAIRS (DoubleRow K=256)
NCH = NT // 512     # 16 token chunks
N_CORES = 8
WSCALE = 64.0       # fp8 weight pre-scale (fold-out: exp scale, W_O)

_cache = {}


def _build_nc():
    from contextlib import ExitStack

    import concourse.mybir as mybir
    import concourse.tile as tile
    from concourse import bacc

    BF = mybir.dt.bfloat16
    F32 = mybir.dt.float32
    EXP = mybir.ActivationFunctionType.Exp

    nc = bacc.Bacc("TRN2", target_bir_lowering=False, debug=False)

    # x chunk-major: [chunk, partition, mt, col]
    xT_d = nc.dram_tensor("xT", [NCH, 128, MT, 512], BF, kind="ExternalInput")
    wq_d = nc.dram_tensor("wqT", [MT, 128, 128], BF, kind="ExternalInput")
    wk_d = nc.dram_tensor("wkT", [MT, 128, 128], BF, kind="ExternalInput")
    wv_d = nc.dram_tensor("wvT", [MT, 128, 128], BF, kind="ExternalInput")
    wo_d = nc.dram_tensor("woT", [128, D], BF, kind="ExternalInput")
    cm_d = nc.dram_tensor("cmask", [128, 2, 128], BF, kind="ExternalInput")
    out_d = nc.dram_tensor("out", [NT, D], BF, kind="ExternalOutput")

    with tile.TileContext(nc) as tc, ExitStack() as ctx:
        pers = ctx.enter_context(tc.tile_pool(name="pers", bufs=1))
        wq = pers.tile([128, MT, 128], BF)
        wk = pers.tile([128, MT, 128], BF)
        wv = pers.tile([128, MT, 128], BF)
        wo = pers.tile([128, D], BF)
        cmask = pers.tile([128, 2, 128], BF)
        ebias = pers.tile([128, 1], F32)

        P = ctx.enter_context
        xpool = P(tc.tile_pool(name="xc", bufs=4))
        qtp = P(tc.tile_pool(name="qt", bufs=3))
        ktp = P(tc.tile_pool(name="kt", bufs=3))
        vbp = P(tc.tile_pool(name="vbp", bufs=3))
        hop = P(tc.tile_pool(name="hop", bufs=3))
        exdp = P(tc.tile_pool(name="exd", bufs=4))
        rvp = P(tc.tile_pool(name="rvp", bufs=2))
        bcast_p = P(tc.tile_pool(name="bcast", bufs=2))
        osbp = P(tc.tile_pool(name="osb", bufs=4))
        p1p = P(tc.tile_pool(name="p1", bufs=1, space="PSUM"))
        popp = P(tc.tile_pool(name="pop", bufs=1, space="PSUM"))
        spp = P(tc.tile_pool(name="sp", bufs=2, space="PSUM"))
        avp = P(tc.tile_pool(name="avp", bufs=1, space="PSUM"))

        state = {"first": True}
        qkv = {}
        hots = {}

        def phase1(b):
            # ---- q/k/v projections for batch b (bf16) ----
            first = state["first"]
            if b not in qkv:
                qt = qtp.tile([128, T], BF, tag="qt", name="qt")
                kt = ktp.tile([128, T], BF, tag="kt", name="kt")
                # [kpos, pair, head, tile-in-pair, 128]; col 0 = ones (so
                # the softmax denominator lands on PSUM partition 0 — the
                # HW reciprocal ucode only honors base partition 0), cols
                # 64-127 = v (64-aligned partition base for the normalize
                # read), cols 1-63 zero
                vb = vbp.tile([128, 8, 2, 2, 128], BF, tag="vb", name="vb")
                nc.vector.memset(vb[:, :, :, :, 0:1], 1.0)
                nc.vector.memset(vb[:, :, :, :, 1:64], 0.0)
                qkv[b] = (qt, kt, vb)
            qt, kt, vb = qkv[b]
            for lc in range(4):
                c = 4 * b + lc
                cs = slice(lc * 512, (lc + 1) * 512)
                xc = xpool.tile([128, MT, 512], BF, tag="xc", name="xc")
                if first:
                    nc.vector.memset(ebias, -1.5)
                    # first chunk: spread DMA issue across idle engines so
                    # the first q matmul can start as soon as wq[0] +
                    # xc[:, 0] land
                    for mt in range(MT):
                        nc.gpsimd.dma_start(out=wq[:, mt, :], in_=wq_d[mt])
                        nc.sync.dma_start(out=xc[:, mt], in_=xT_d[c, :, mt])
                    for mt in range(MT):
                        nc.gpsimd.dma_start(out=wk[:, mt, :], in_=wk_d[mt])
                        nc.scalar.dma_start(out=wv[:, mt, :], in_=wv_d[mt])
                    nc.scalar.dma_start(out=wo[:], in_=wo_d[:])
                    nc.scalar.dma_start(out=cmask, in_=cm_d[:])
                    first = state["first"] = False
                else:
                    nc.sync.dma_start(out=xc, in_=xT_d[c])
                for name, wt, dst in (("pq", wq, qt), ("pk", wk, kt)):
                    p = p1p.tile([128, 512], F32, tag="p1", name=name)
                    for mt in range(MT):
                        nc.tensor.matmul(p, wt[:, mt, :], xc[:, mt],
                                         start=(mt == 0), stop=(mt == MT - 1))
                    nc.vector.tensor_copy(out=dst[:, cs], in_=p)
                pv = p1p.tile([128, 4, 128], F32, tag="p1", name="pv")
                for tt in range(4):
                    for mt in range(MT):
                        nc.tensor.matmul(pv[:, tt, :],
                                         xc[:, mt, tt * 128:(tt + 1) * 128],
                                         wv[:, mt, :],
                                         start=(mt == 0), stop=(mt == MT - 1))
                # vb[:, 2lc+a, h, b2, 64:128] <- pv[:, 2a+b2, 64h:64h+64]
                for h in range(2):
                    nc.vector.tensor_copy(
                        out=vb[:, 2 * lc:2 * lc + 2, h, :, 64:128],
                        in_=pv[:, :, 64 * h:64 * h + 64])

        def emit_proj(b, hoT, qc):
            # output projection for one q-chunk
            for tt in range(4 * qc, 4 * qc + 4):
                osb = osbp.tile([128, D], BF, tag="osb", name="osb")
                for oc in range(2):
                    po = popp.tile([128, 512], F32, tag="po", name="po")
                    nc.tensor.matmul(
                        po, hoT[:, tt * 128:(tt + 1) * 128],
                        wo[:, oc * 512:(oc + 1) * 512],
                        start=True, stop=True)
                    nc.vector.tensor_copy(
                        out=osb[:, oc * 512:(oc + 1) * 512], in_=po)
                to = b * T + tt * 128
                nc.sync.dma_start(out=out_d[to:to + 128, :], in_=osb)

        def attn(b):
            # ---- causal attention (bf16) + projection ----
            # PE stream is software-pipelined: the AV matmuls for k-tile
            # kt are emitted after the scores for kt+1 (so the PE never
            # sits behind the ~1.1us ACT exp latency), and the output
            # projection for q-chunk qc is emitted after the AV block of
            # qc+1 (hiding the normalize chain and the pav PSUM-ring
            # reuse stall).
            qt, kt, vb = qkv[b]
            if b not in hots:
                hots[b] = hop.tile([128, T], BF, tag="hoT", name="hoT")
            hoT = hots[b]
            pend_proj = None
            for qc in range(4):
                q0 = qc * 512
                nk = 4 * qc + 4
                pavs = [avp.tile([128, 512], F32, tag=f"pav{h}",
                                 name=f"pav{h}") for h in range(2)]
                pend_av = None
                for kt_i in range(nk):
                    off = 128 * (kt_i - 4 * qc) if kt_i >= 4 * qc else 0
                    pss = spp.tile([128, 2, 512], F32, tag="pss", name="pss")
                    for h in range(2):
                        hp = 64 * h
                        nc.tensor.matmul(
                            pss[:, h, off:512],
                            kt[hp:hp + 64, kt_i * 128:(kt_i + 1) * 128],
                            qt[hp:hp + 64, q0 + off:q0 + 512],
                            start=True, stop=True)
                    exd = exdp.tile([128, 2, 512], BF, tag="exd", name="exd")
                    nc.scalar.activation(out=exd[:, :, off:512],
                                         in_=pss[:, :, off:512],
                                         func=EXP, scale=0.125, bias=ebias)
                    if kt_i >= 4 * qc:
                        nc.vector.tensor_mul(exd[:, :, off:off + 128],
                                             exd[:, :, off:off + 128], cmask)
                    if pend_av is not None:
                        pkt, pexd, poff = pend_av
                        for h in range(2):
                            nc.tensor.matmul(
                                pavs[h][:, poff:512],
                                vb[:, pkt // 2, h, pkt % 2],
                                pexd[:, h, poff:512],
                                start=(pkt == 0), stop=False,
                                skip_group_check=True)
                    pend_av = (kt_i, exd, off)
                pkt, pexd, poff = pend_av
                for h in range(2):
                    nc.tensor.matmul(
                        pavs[h][:, poff:512],
                        vb[:, pkt // 2, h, pkt % 2],
                        pexd[:, h, poff:512],
                        start=(pkt == 0), stop=True,
                        skip_group_check=True)
                # chunk tail: 1/denominator (PSUM row 0), broadcast,
                # normalize into hoT
                for h in range(2):
                    rv = rvp.tile([1, 512], F32, tag="rv", name="rv")
                    nc.vector.reciprocal_approx_fast(out=rv,
                                                     in_=pavs[h][0:1, :])
                    invb = bcast_p.tile([64, 512], F32, tag=f"invb{h}",
                                        name=f"invb{h}")
                    nc.gpsimd.partition_broadcast(invb, rv)
                    nc.vector.tensor_mul(
                        hoT[64 * h:64 * h + 64, q0:q0 + 512],
                        pavs[h][64:128, :], invb)
                if pend_proj is not None:
                    emit_proj(b, hoT, pend_proj)
                pend_proj = qc
            emit_proj(b, hoT, pend_proj)

        for b in range(B):
            phase1(b)
            attn(b)
    nc.compile()
    return nc


def _get_nc():
    if "nc" not in _cache:
        _cache["nc"] = _build_nc()
    return _cache["nc"]


def _bf(a):
    return np.ascontiguousarray(a, dtype=np.float32).astype(ml_dtypes.bfloat16)


def _f8(a):
    return np.ascontiguousarray(a, dtype=np.float32).astype(ml_dtypes.float8_e4m3)


def make_in_maps(x, W_Q, W_K, W_V, W_O):
    xT = _bf(x.reshape(NT, D).T)                      # [D, NT]
    # [chunk, partition, mt, col]
    xTc = np.ascontiguousarray(
        xT.reshape(MT, 128, NCH, 512).transpose(2, 1, 0, 3))
    cmask = np.ones((128, 2, 128), dtype=np.float32)
    for kp in range(128):
        cmask[kp, :, :kp] = 0.0
    cmask = cmask.astype(ml_dtypes.bfloat16)
    in_maps = []
    for c in range(N_CORES):
        rs = slice(c * 128, (c + 1) * 128)
        in_maps.append({
            "xT": xTc,
            "wqT": _bf(W_Q[rs, :].T).reshape(MT, 128, 128),
            "wkT": _bf(W_K[rs, :].T).reshape(MT, 128, 128),
            "wvT": _bf(W_V[rs, :].T).reshape(MT, 128, 128),
            "woT": _bf(W_O[:, rs].T),
            "cmask": cmask,
        })
    return in_maps


def _ensure_ntff_hook():
    """Install antenv.axon_hooks shim (missing in this image) so
    run_bass_kernel_spmd(trace=True) can capture NTFF profiles."""
    try:
        from antenv import axon_hooks  # noqa: F401
        return True
    except ImportError:
        pass
    try:
        import contextlib
        import ctypes
        import types

        import antenv

        so_path = "/opt/axon/libaxon_pjrt.so"
        lib = ctypes.CDLL(so_path)
        if not hasattr(lib, "axon_start_nrt_profile"):
            return False
        lib.axon_start_nrt_profile.argtypes = [
            ctypes.POINTER(ctypes.c_int64), ctypes.c_size_t]
        lib.axon_start_nrt_profile.restype = ctypes.c_int64
        lib.axon_stop_nrt_profile.argtypes = [ctypes.c_char_p]
        lib.axon_stop_nrt_profile.restype = ctypes.c_int64

        @contextlib.contextmanager
        def _hook(output_dir, device_ids):
            import jax
            jax.devices()
            if device_ids:
                ids = (ctypes.c_int64 * len(device_ids))(*device_ids)
                rc = lib.axon_start_nrt_profile(ids, len(device_ids))
            else:
                rc = lib.axon_start_nrt_profile(None, 0)
            if rc != 0:
                raise RuntimeError(f"axon_start_nrt_profile rc={rc}")
            try:
                yield
            finally:
                n = lib.axon_stop_nrt_profile(str(output_dir).encode())
                print(f"ntff profile: {n} file(s) -> {output_dir}",
                      file=sys.stderr)

        mod = types.ModuleType("antenv.axon_hooks")
        mod.get_axon_ntff_profile_hook = lambda: _hook
        mod.set_axon_ntff_profile_hook = lambda h: None
        sys.modules["antenv.axon_hooks"] = mod
        antenv.axon_hooks = mod
        return True
    except Exception as e:  # pragma: no cover
        print(f"ntff hook install failed: {e}", file=sys.stderr)
        return False


def run_on_cores(in_maps, trace=False, trace_all_cores=False):
    """Compile once, run on cores 0..7; optional NTFF profiling."""
    import concourse.bass_utils as bass_utils

    nc = _get_nc()
    tmpdir = None
    trace_cores = None
    if trace:
        trace = _ensure_ntff_hook()
    if trace:
        import tempfile
        tmpdir = tempfile.mkdtemp(prefix="mhsa_ntff_")
        _cache["trace_dir"] = tmpdir
        # no cloud creds in this container; keep artifacts local
        bass_utils.upload_artifacts = lambda d: f"local://{d}"
        if trace_all_cores:
            trace_cores = list(range(N_CORES))
    res = bass_utils.run_bass_kernel_spmd(
        nc, in_maps, list(range(N_CORES)), trace=trace, tmpdir=tmpdir,
        trace_cores=trace_cores)
    _cache["last_results"] = res
    return res


def kernel(x, W_Q, W_K, W_V, W_O):
    x = np.asarray(x, dtype=np.float32)
    in_maps = make_in_maps(x, np.asarray(W_Q, np.float32),
                           np.asarray(W_K, np.float32),
                           np.asarray(W_V, np.float32),
                           np.asarray(W_O, np.float32))
    trace = bool(int(os.environ.get("MHSA_TRACE", "0")))
    all_cores = bool(int(os.environ.get("MHSA_TRACE_ALL_CORES", "0")))
    res = run_on_cores(in_maps, trace=trace, trace_all_cores=all_cores)
    out = np.zeros((NT, D), dtype=np.float32)
    for r in res.results:
        out += np.asarray(r["out"], dtype=np.float32)
    return out.reshape(B, T, D)
